# revision 22
# baseline (speedup 1.0000x reference)
"""Trainium2 Bass kernel: DeepSeek-style MoE layer (16 experts, top-2).

Strategy (expert-parallel, 8 cores):
  - Host computes the router (softmax + top-2 + renorm) in fp32 numpy and
    builds the token dispatch.  Experts are paired large-with-small onto
    cores; slot A holds up to 1152 tokens (9 tiles), slot B up to 1024
    (8 tiles).  Gathered tokens ship transposed ([D, slots]) in fp8-e4m3
    (scaled by SX), partition-major so every DMA row is one contiguous
    descriptor.
  - Device (per core, identical SPMD program), all matmuls fp8 DoubleRow
    (two 128-row contractions per instruction = 2x PE throughput):
        gT/uT = w13.T @ xT          (PSUM fp32 = S1 * true, [H_tile, tok])
        sg    = silu(gT / S1)       (scalar engine, bf16)
        hT    = sg * uT             (fp8, = S1 * h_true)
        y     = hT.T @ w2           (PSUM fp32 = S1*SW2 * true)
        ys    = y * wg'             (wg' = combine_weight / (S1*SW2))
    Stage-2 combine alternates vector/scalar engines to balance load.
  - Host scatter-adds the (already weighted) expert outputs into the
    residual stream.

Hardcoded for B=2, S=4096, D=1024, H=512, E=16, K=2.
"""

import numpy as np

B, S, D, H, E, TOPK = 2, 4096, 1024, 512, 16, 2
T = B * S
NCORES = 8
EPC = E // NCORES          # experts per core = 2
SLOT_CAP = [1152, 1024]    # token capacity per slot (A, B)
SLOT_OFF = [0, 1152]
TOT = sum(SLOT_CAP)        # 2176 token slots per core
NTT = TOT // 128           # 17 token tiles per core
# (slot, token offset within slot, length) — one xt DMA slab per entry.
# The small 128-token block runs LAST so the post-matmul tail drains only
# one tile's combine+DMA.
XBLOCKS = [(0, 0, 512), (0, 512, 512), (1, 0, 512),
           (1, 512, 512), (0, 1024, 128)]
ND = D // 128              # 8 d-tiles (stage-1 contraction)
NH = H // 128              # 4 h-tiles
H2 = 2 * H                 # w1|w3 fused column width

# fp8 scaling: x*SX and w13*SW1 keep operands inside e4m3's normal range
# (w has std 0.02, below e4m3's 2^-6 min normal unscaled).
SX = 2.0
SW1 = 8.0
SW2 = 8.0
S1 = SX * SW1              # scale of stage-1 PSUM (g, u)
S2 = S1 * SW2              # scale of stage-2 PSUM (y)
FP8_MAX = 240.0            # TRN e4m3 max normal

_PROG = None
_LAST_RESULTS = None


def _pair(ap):
    """[128, 2*c] AP -> [128, 2, c] view for DoubleRow matmul operands."""
    return ap.rearrange("p (two c) -> p two c", two=2)


def _build_program():
    import concourse.bacc as bacc
    import concourse.tile as tile
    from concourse import mybir

    BF = mybir.dt.bfloat16
    FP8 = mybir.dt.float8e4
    F32 = mybir.dt.float32
    AF = mybir.ActivationFunctionType
    DR = mybir.MatmulPerfMode.DoubleRow

    nc = bacc.Bacc("TRN2", target_bir_lowering=False, debug=False,
                   num_devices=NCORES)

    # DRAM I/O (per core), all partition-major: row p holds everything
    # partition p will need, contiguously.
    xt_ds = [nc.dram_tensor(f"xtb{bi}", [128, ND * n], FP8,
                            kind="ExternalInput")
             for bi, (_, _, n) in enumerate(XBLOCKS)]
    w13_d = nc.dram_tensor("w13", [EPC, 128, ND * H2], FP8,
                           kind="ExternalInput")
    w2_d = nc.dram_tensor("w2", [EPC, 128, NH * D], FP8, kind="ExternalInput")
    wg_d = nc.dram_tensor("wg", [128, NTT], F32, kind="ExternalInput")
    y_d = nc.dram_tensor("y", [NTT, 128, D], BF, kind="ExternalOutput")
    # scratch sink for the DMA-ordering gates (see dispatch section)
    gate_d = nc.dram_tensor("gate_scratch", [128, 128], FP8, kind="Internal")

    w13 = w13_d.ap()
    w2 = w2_d.ap()
    wg = wg_d.ap()
    y = y_d.ap()

    with tile.TileContext(nc) as tc:
        with (
            tc.tile_pool(name="wpool", bufs=1) as wpool,
            tc.tile_pool(name="hpool", bufs=2) as hpool,
            tc.tile_pool(name="ypool", bufs=6) as ypool,
            tc.tile_pool(name="ps1", bufs=2, space="PSUM") as ps1,
            tc.tile_pool(name="ps2", bufs=4, space="PSUM") as ps2,
        ):
            # ---- HAM warmup: dummy matmuls on a zeroed scratch tile so the
            # PE clock-gate opens while input DMAs stream in.
            warm = wpool.tile([128, 512], BF, tag="warm", name="warm")
            nc.vector.memset(warm[:], 0.0)
            wps = ps1.tile([128, 512], F32, tag="g", name="wps")
            for i in range(5):
                nc.tensor.matmul(wps[:], warm[:, 0:128], warm[:],
                                 start=(i == 0), stop=(i == 4))

            # ---- static SBUF-resident inputs ----
            wg_sb = wpool.tile([128, NTT], F32, tag="wg", name="wg")
            xt_sb = [wpool.tile([128, ND * n], FP8, tag=f"xtb{bi}",
                                name=f"xtb{bi}")
                     for bi, (_, _, n) in enumerate(XBLOCKS)]
            # free layout: (ht, w) major, then dt, then 128 cols.  One tile
            # per (expert, ht) chunk so a stage-1 group only waits for its
            # own ht slab, not the whole expert weight DMA.
            w13_sb = [[wpool.tile([128, 2 * ND * 128], FP8,
                                  tag=f"w13_{e}_{ht}", name=f"w13_{e}_{ht}")
                       for ht in range(NH)] for e in range(EPC)]
            w2_sb = [wpool.tile([128, NH * D], FP8, tag=f"w2_{e}",
                                name=f"w2_{e}") for e in range(EPC)]

            # DMA issue order = need order.  The first real matmul needs ALL
            # of xtb0 + w13[ht0]: dispatch those three slabs in parallel on
            # all three DMA-capable rings (sync, scalar, gpsimd), then stream
            # the rest on sync/scalar.  gpsimd afterwards carries wg + all y
            # output DMAs so input dispatch never queues behind outputs.
            HTC = 2 * ND * 128           # columns per ht-chunk of w13
            XH = ND * 512 // 2
            # Critical set first: xtb0 halves + w13[0][ht0].  In-flight
            # transfers share the 16 HW queues round-robin, so everything
            # else is held back by a gate DMA that reads xt_sb[0] — it can
            # only dispatch once xtb0 is fully landed, keeping the critical
            # path at full bandwidth.
            nc.sync.dma_start(xt_sb[0][:, 0:XH], xt_ds[0].ap()[:, 0:XH])
            nc.scalar.dma_start(xt_sb[0][:, XH:], xt_ds[0].ap()[:, XH:])
            nc.sync.dma_start(w13_sb[0][0][:], w13[0, :, 0:HTC])
            ga = gate_d.ap()
            nc.sync.dma_start(ga[:, 0:64], xt_sb[0][:, 0:64])
            nc.scalar.dma_start(ga[:, 64:128], xt_sb[0][:, 64:128])
            nc.scalar.dma_start(w13_sb[0][1][:], w13[0, :, HTC:2 * HTC])
            nc.sync.dma_start(w13_sb[0][2][:], w13[0, :, 2 * HTC:3 * HTC])
            nc.scalar.dma_start(w13_sb[0][3][:], w13[0, :, 3 * HTC:4 * HTC])
            nc.sync.dma_start(xt_sb[1][:], xt_ds[1].ap()[:])
            nc.scalar.dma_start(w2_sb[0][:], w2[0])
            nc.sync.dma_start(xt_sb[2][:], xt_ds[2].ap()[:])
            for ht in range(NH):
                ring = nc.scalar if ht % 2 == 0 else nc.sync
                ring.dma_start(w13_sb[1][ht][:],
                               w13[1, :, ht * HTC:(ht + 1) * HTC])
            nc.sync.dma_start(xt_sb[3][:], xt_ds[3].ap()[:])
            nc.scalar.dma_start(w2_sb[1][:], w2[1])
            nc.sync.dma_start(xt_sb[4][:], xt_ds[4].ap()[:])
            nc.gpsimd.dma_start(wg_sb[:], wg[:])
            yring = [nc.sync, nc.gpsimd]

            # ---- compute ----
            for bi, (e, off, n) in enumerate(XBLOCKS):
                h_all = hpool.tile([128, NH * 512], FP8, tag="h", name="h")
                for ht in range(NH):
                    g = ps1.tile([128, 512], F32, tag="g", name="g")
                    u = ps1.tile([128, 512], F32, tag="u", name="u")
                    for w in range(2):
                        dst = g if w == 0 else u
                        for j in range(ND // 2):
                            o = (w * ND + 2 * j) * 128
                            nc.tensor.matmul(
                                dst[:, :n],
                                _pair(w13_sb[e][ht][:, o: o + 256]),
                                _pair(xt_sb[bi][:, 2 * j * n: (2 * j + 2) * n]),
                                start=(j == 0), stop=(j == ND // 2 - 1),
                                perf_mode=DR,
                            )
                    sg = hpool.tile([128, 512], BF, tag="sg", name="sg")
                    nc.scalar.activation(sg[:, :n], g[:, :n], AF.Silu,
                                         scale=1.0 / S1)
                    nc.vector.tensor_mul(h_all[:, ht * 512: ht * 512 + n],
                                         sg[:, :n], u[:, :n])
                for tt in range(n // 128):
                    gtt = (SLOT_OFF[e] + off) // 128 + tt
                    ys = ypool.tile([128, D], BF, tag="ys", name="ys")
                    for db in range(2):
                        yp = ps2.tile([128, 512], F32, tag="yp", name="yp")
                        for hp in range(NH // 2):
                            nc.tensor.matmul(
                                yp[:],
                                _pair(h_all[:, 2 * hp * 512:
                                            (2 * hp + 2) * 512])
                                [:, :, tt * 128:(tt + 1) * 128],
                                _pair(w2_sb[e][:, 2 * hp * D:
                                               (2 * hp + 2) * D])
                                [:, :, db * 512:(db + 1) * 512],
                                start=(hp == 0), stop=(hp == NH // 2 - 1),
                                perf_mode=DR,
                            )
                        if db == 0:
                            nc.vector.tensor_scalar_mul(
                                ys[:, 0:512], yp[:], wg_sb[:, gtt:gtt + 1])
                        else:
                            nc.scalar.mul(ys[:, 512:1024], yp[:],
                                          wg_sb[:, gtt:gtt + 1])
                    yring[gtt % 2].dma_start(y[gtt], ys[:])

    nc.compile()
    return nc


def _program():
    global _PROG
    if _PROG is None:
        _PROG = _build_program()
    return _PROG


def _route(x, gate_w):
    """fp32 softmax router + top-2 with renormalized weights (matches ref)."""
    logits = x @ gate_w.astype(np.float32)
    logits = logits - logits.max(axis=-1, keepdims=True)
    ex = np.exp(logits)
    scores = ex / ex.sum(axis=-1, keepdims=True)
    idx = np.argsort(-scores, axis=-1, kind="stable")[:, :TOPK]
    w = np.take_along_axis(scores, idx, axis=-1)
    w = w / w.sum(axis=-1, keepdims=True)
    return idx, w.astype(np.float32)


def _moe_numpy(x, gate_w, w1, w3, w2):
    """Slow exact fallback (only used if a capacity overflow ever happens)."""
    idx, wts = _route(x, gate_w)
    out = x.copy()
    for e in range(E):
        sel = np.nonzero(idx == e)
        toks = sel[0]
        ww = wts[sel]
        xe = x[toks]
        g = xe @ w1[e]
        u = xe @ w3[e]
        h = (g / (1.0 + np.exp(-g))) * u
        out[toks] += (h @ w2[e]) * ww[:, None]
    return out


def _quant_fp8(a, scale):
    import ml_dtypes
    return np.clip(a * scale, -FP8_MAX, FP8_MAX).astype(ml_dtypes.float8_e4m3)


def _pack_w13(a):
    """[D, 2H] -> [128, ND*2H], columns ordered (ht, w1|w3, dt, 128)."""
    r = a.reshape(ND, 128, 2, NH, 128)        # dt, p, w, ht, c
    r = r.transpose(1, 3, 2, 0, 4)            # p, ht, w, dt, c
    return np.ascontiguousarray(r.reshape(128, ND * H2))


def _pmajor(a, cols):
    """[rows=nd*128, cols] -> [128, nd*cols] partition-major layout."""
    nd = a.shape[0] // 128
    return np.ascontiguousarray(
        a.reshape(nd, 128, cols).transpose(1, 0, 2).reshape(128, nd * cols))


def kernel(hidden_states, gate_w, w1, w3, w2):
    from concourse import bass_utils

    hidden_states = np.asarray(hidden_states, dtype=np.float32)
    gate_w = np.asarray(gate_w, dtype=np.float32)
    w1 = np.asarray(w1, dtype=np.float32)
    w3 = np.asarray(w3, dtype=np.float32)
    w2 = np.asarray(w2, dtype=np.float32)

    x = hidden_states.reshape(T, D)
    idx, wts = _route(x, gate_w)

    tok_lists = []
    wt_lists = []
    for e in range(E):
        sel = np.nonzero(idx == e)
        tok_lists.append(sel[0])
        wt_lists.append(wts[sel])
    counts = np.array([len(t) for t in tok_lists])

    # pair largest with smallest; slot A = larger of the pair
    order = np.argsort(-counts, kind="stable")
    pairs = [(order[i], order[E - 1 - i]) for i in range(NCORES)]
    if any(counts[a] > SLOT_CAP[0] or counts[b] > SLOT_CAP[1]
           for a, b in pairs):
        return _moe_numpy(x, gate_w, w1, w3, w2).reshape(B, S, D)

    xq = _quant_fp8(x, SX)                                    # [T, D] fp8
    w13q = _quant_fp8(np.concatenate([w1, w3], axis=2), SW1)  # [E, D, 2H]
    w13q = np.stack([_pack_w13(w13q[e]) for e in range(E)])
    w2q = _quant_fp8(w2, SW2)
    w2q = np.stack([_pmajor(w2q[e], D) for e in range(E)])

    in_maps = []
    for c in range(NCORES):
        xg = np.zeros((TOT, D), dtype=xq.dtype)
        wgt = np.zeros(TOT, dtype=np.float32)
        for j, e in enumerate(pairs[c]):
            ne = counts[e]
            xg[SLOT_OFF[j]:SLOT_OFF[j] + ne] = xq[tok_lists[e]]
            wgt[SLOT_OFF[j]:SLOT_OFF[j] + ne] = wt_lists[e] / S2
        xgT = np.ascontiguousarray(xg.T)       # [D, TOT]
        ea, eb = pairs[c]
        m = {
            "w13": np.stack([w13q[ea], w13q[eb]]),
            "w2": np.stack([w2q[ea], w2q[eb]]),
            "wg": np.ascontiguousarray(wgt.reshape(NTT, 128).T),
        }
        for bi, (s, off, n) in enumerate(XBLOCKS):
            c0 = SLOT_OFF[s] + off
            m[f"xtb{bi}"] = _pmajor(xgT[:, c0:c0 + n], n)
        in_maps.append(m)

    res = bass_utils.run_bass_kernel_spmd(
        _program(), in_maps, core_ids=list(range(NCORES)))
    global _LAST_RESULTS
    _LAST_RESULTS = res

    out = x.copy()
    for c in range(NCORES):
        yc = np.asarray(res.results[c]["y"], dtype=np.float32)
        yc = yc.reshape(NTT * 128, D)
        for j, e in enumerate(pairs[c]):
            ne = counts[e]
            out[tok_lists[e]] += yc[SLOT_OFF[j]:SLOT_OFF[j] + ne]
    return out.reshape(B, S, D)


# revision 23
# speedup vs baseline: 1.1056x; 1.1056x over previous
"""Trainium2 Bass kernel: DeepSeek-style MoE layer (16 experts, top-2).

Strategy (expert-parallel, 8 cores):
  - Host computes the router (softmax + top-2 + renorm) in fp32 numpy and
    builds the token dispatch.  Experts are paired large-with-small onto
    cores; slot A holds up to 1152 tokens (9 tiles), slot B up to 1024
    (8 tiles).  Gathered tokens ship transposed ([D, slots]) in fp8-e4m3
    (scaled by SX), partition-major so every DMA row is one contiguous
    descriptor.
  - Device (per core, identical SPMD program), all matmuls fp8 DoubleRow
    (two 128-row contractions per instruction = 2x PE throughput):
        gT/uT = w13.T @ xT          (PSUM fp32 = S1 * true, [H_tile, tok])
        sg    = silu(gT / S1)       (scalar engine, bf16)
        hT    = sg * uT             (fp8, = S1 * h_true)
        y     = hT.T @ w2           (PSUM fp32 = S1*SW2 * true)
        ys    = y * wg'             (wg' = combine_weight / (S1*SW2))
    Stage-2 combine alternates vector/scalar engines to balance load.
  - Host scatter-adds the (already weighted) expert outputs into the
    residual stream.

Hardcoded for B=2, S=4096, D=1024, H=512, E=16, K=2.
"""

import numpy as np

B, S, D, H, E, TOPK = 2, 4096, 1024, 512, 16, 2
T = B * S
NCORES = 8
EPC = E // NCORES          # experts per core = 2
SLOT_CAP = [1152, 1024]    # token capacity per slot (A, B)
SLOT_OFF = [0, 1152]
TOT = sum(SLOT_CAP)        # 2176 token slots per core
NTT = TOT // 128           # 17 token tiles per core
# (slot, token offset within slot, length) — one xt DMA slab per entry.
# The small 128-token block runs LAST so the post-matmul tail drains only
# one tile's combine+DMA.
XBLOCKS = [(0, 0, 512), (0, 512, 512), (1, 0, 512),
           (1, 512, 512), (0, 1024, 128)]
ND = D // 128              # 8 d-tiles (stage-1 contraction)
NH = H // 128              # 4 h-tiles
H2 = 2 * H                 # w1|w3 fused column width

# fp8 scaling: x*SX and w13*SW1 keep operands inside e4m3's normal range
# (w has std 0.02, below e4m3's 2^-6 min normal unscaled).
SX = 2.0
SW1 = 8.0
SW2 = 8.0
S1 = SX * SW1              # scale of stage-1 PSUM (g, u)
S2 = S1 * SW2              # scale of stage-2 PSUM (y)
FP8_MAX = 240.0            # TRN e4m3 max normal

_PROG = None
_LAST_RESULTS = None


def _pair(ap):
    """[128, 2*c] AP -> [128, 2, c] view for DoubleRow matmul operands."""
    return ap.rearrange("p (two c) -> p two c", two=2)


def _build_program():
    import concourse.bacc as bacc
    import concourse.tile as tile
    from concourse import mybir

    BF = mybir.dt.bfloat16
    FP8 = mybir.dt.float8e4
    F32 = mybir.dt.float32
    AF = mybir.ActivationFunctionType
    DR = mybir.MatmulPerfMode.DoubleRow

    nc = bacc.Bacc("TRN2", target_bir_lowering=False, debug=False,
                   num_devices=NCORES)

    # DRAM I/O (per core), all partition-major: row p holds everything
    # partition p will need, contiguously.
    xt_ds = [nc.dram_tensor(f"xtb{bi}", [128, ND * n], FP8,
                            kind="ExternalInput")
             for bi, (_, _, n) in enumerate(XBLOCKS)]
    w13_d = nc.dram_tensor("w13", [EPC, 128, ND * H2], FP8,
                           kind="ExternalInput")
    w2_d = nc.dram_tensor("w2", [EPC, 128, NH * D], FP8, kind="ExternalInput")
    wg_d = nc.dram_tensor("wg", [128, NTT], F32, kind="ExternalInput")
    y_d = nc.dram_tensor("y", [NTT, 128, D], BF, kind="ExternalOutput")
    # scratch sink for the DMA-ordering gates (see dispatch section)
    gate_d = nc.dram_tensor("gate_scratch", [128, 128], FP8, kind="Internal")

    w13 = w13_d.ap()
    w2 = w2_d.ap()
    wg = wg_d.ap()
    y = y_d.ap()

    with tile.TileContext(nc) as tc:
        with (
            tc.tile_pool(name="wpool", bufs=1) as wpool,
            tc.tile_pool(name="hpool", bufs=2) as hpool,
            tc.tile_pool(name="ypool", bufs=6) as ypool,
            tc.tile_pool(name="ps1", bufs=2, space="PSUM") as ps1,
            tc.tile_pool(name="ps2", bufs=4, space="PSUM") as ps2,
        ):
            # ---- HAM warmup: dummy matmuls on a zeroed scratch tile so the
            # PE clock-gate opens while input DMAs stream in.
            warm = wpool.tile([128, 512], BF, tag="warm", name="warm")
            nc.vector.memset(warm[:], 0.0)
            wps = ps1.tile([128, 512], F32, tag="g", name="wps")
            for i in range(8):
                nc.tensor.matmul(wps[:], warm[:, 0:128], warm[:],
                                 start=(i == 0), stop=(i == 7))

            # ---- static SBUF-resident inputs ----
            wg_sb = wpool.tile([128, NTT], F32, tag="wg", name="wg")
            xt_sb = [wpool.tile([128, ND * n], FP8, tag=f"xtb{bi}",
                                name=f"xtb{bi}")
                     for bi, (_, _, n) in enumerate(XBLOCKS)]
            # free layout: (ht, w) major, then dt, then 128 cols.  One tile
            # per (expert, ht) chunk so a stage-1 group only waits for its
            # own ht slab, not the whole expert weight DMA.
            w13_sb = [[wpool.tile([128, 2 * ND * 128], FP8,
                                  tag=f"w13_{e}_{ht}", name=f"w13_{e}_{ht}")
                       for ht in range(NH)] for e in range(EPC)]
            w2_sb = [wpool.tile([128, NH * D], FP8, tag=f"w2_{e}",
                                name=f"w2_{e}") for e in range(EPC)]

            # DMA issue order = need order.  The first real matmul needs ALL
            # of xtb0 + w13[ht0]: dispatch those three slabs in parallel on
            # all three DMA-capable rings (sync, scalar, gpsimd), then stream
            # the rest on sync/scalar.  gpsimd afterwards carries wg + all y
            # output DMAs so input dispatch never queues behind outputs.
            HTC = 2 * ND * 128           # columns per ht-chunk of w13
            XH = ND * 512 // 2
            # Critical set first: xtb0 halves + w13[0][ht0].  In-flight
            # transfers share the 16 HW queues round-robin, so everything
            # else is held back by a gate DMA that reads xt_sb[0] — it can
            # only dispatch once xtb0 is fully landed, keeping the critical
            # path at full bandwidth.
            nc.sync.dma_start(xt_sb[0][:, 0:XH], xt_ds[0].ap()[:, 0:XH])
            nc.scalar.dma_start(xt_sb[0][:, XH:], xt_ds[0].ap()[:, XH:])
            nc.sync.dma_start(w13_sb[0][0][:], w13[0, :, 0:HTC])
            ga = gate_d.ap()
            nc.sync.dma_start(ga[:, 0:64], xt_sb[0][:, 0:64])
            nc.scalar.dma_start(ga[:, 64:128], xt_sb[0][:, 64:128])
            nc.scalar.dma_start(w13_sb[0][1][:], w13[0, :, HTC:2 * HTC])
            nc.sync.dma_start(w13_sb[0][2][:], w13[0, :, 2 * HTC:3 * HTC])
            nc.scalar.dma_start(w13_sb[0][3][:], w13[0, :, 3 * HTC:4 * HTC])
            nc.sync.dma_start(xt_sb[1][:], xt_ds[1].ap()[:])
            nc.scalar.dma_start(w2_sb[0][:], w2[0])
            nc.sync.dma_start(xt_sb[2][:], xt_ds[2].ap()[:])
            for ht in range(NH):
                ring = nc.scalar if ht % 2 == 0 else nc.sync
                ring.dma_start(w13_sb[1][ht][:],
                               w13[1, :, ht * HTC:(ht + 1) * HTC])
            nc.sync.dma_start(xt_sb[3][:], xt_ds[3].ap()[:])
            nc.scalar.dma_start(w2_sb[1][:], w2[1])
            nc.sync.dma_start(xt_sb[4][:], xt_ds[4].ap()[:])
            nc.gpsimd.dma_start(wg_sb[:], wg[:])
            yring = [nc.sync, nc.gpsimd]

            # ---- compute ----
            for bi, (e, off, n) in enumerate(XBLOCKS):
                h_all = hpool.tile([128, NH * 512], FP8, tag="h", name="h")
                for ht in range(NH):
                    g = ps1.tile([128, 512], F32, tag="g", name="g")
                    u = ps1.tile([128, 512], F32, tag="u", name="u")
                    for w in range(2):
                        dst = g if w == 0 else u
                        for j in range(ND // 2):
                            o = (w * ND + 2 * j) * 128
                            nc.tensor.matmul(
                                dst[:, :n],
                                _pair(w13_sb[e][ht][:, o: o + 256]),
                                _pair(xt_sb[bi][:, 2 * j * n: (2 * j + 2) * n]),
                                start=(j == 0), stop=(j == ND // 2 - 1),
                                perf_mode=DR,
                            )
                    sg = hpool.tile([128, 512], BF, tag="sg", name="sg")
                    nc.scalar.activation(sg[:, :n], g[:, :n], AF.Silu,
                                         scale=1.0 / S1)
                    nc.vector.tensor_mul(h_all[:, ht * 512: ht * 512 + n],
                                         sg[:, :n], u[:, :n])
                for tt in range(n // 128):
                    gtt = (SLOT_OFF[e] + off) // 128 + tt
                    ys = ypool.tile([128, D], BF, tag="ys", name="ys")
                    for db in range(2):
                        yp = ps2.tile([128, 512], F32, tag="yp", name="yp")
                        for hp in range(NH // 2):
                            nc.tensor.matmul(
                                yp[:],
                                _pair(h_all[:, 2 * hp * 512:
                                            (2 * hp + 2) * 512])
                                [:, :, tt * 128:(tt + 1) * 128],
                                _pair(w2_sb[e][:, 2 * hp * D:
                                               (2 * hp + 2) * D])
                                [:, :, db * 512:(db + 1) * 512],
                                start=(hp == 0), stop=(hp == NH // 2 - 1),
                                perf_mode=DR,
                            )
                        if db == 0:
                            nc.vector.tensor_scalar_mul(
                                ys[:, 0:512], yp[:], wg_sb[:, gtt:gtt + 1])
                        else:
                            nc.scalar.mul(ys[:, 512:1024], yp[:],
                                          wg_sb[:, gtt:gtt + 1])
                    yring[gtt % 2].dma_start(y[gtt], ys[:])

    nc.compile()
    return nc


def _program():
    global _PROG
    if _PROG is None:
        _PROG = _build_program()
    return _PROG


def _route(x, gate_w):
    """fp32 softmax router + top-2 with renormalized weights (matches ref)."""
    logits = x @ gate_w.astype(np.float32)
    logits = logits - logits.max(axis=-1, keepdims=True)
    ex = np.exp(logits)
    scores = ex / ex.sum(axis=-1, keepdims=True)
    idx = np.argsort(-scores, axis=-1, kind="stable")[:, :TOPK]
    w = np.take_along_axis(scores, idx, axis=-1)
    w = w / w.sum(axis=-1, keepdims=True)
    return idx, w.astype(np.float32)


def _moe_numpy(x, gate_w, w1, w3, w2):
    """Slow exact fallback (only used if a capacity overflow ever happens)."""
    idx, wts = _route(x, gate_w)
    out = x.copy()
    for e in range(E):
        sel = np.nonzero(idx == e)
        toks = sel[0]
        ww = wts[sel]
        xe = x[toks]
        g = xe @ w1[e]
        u = xe @ w3[e]
        h = (g / (1.0 + np.exp(-g))) * u
        out[toks] += (h @ w2[e]) * ww[:, None]
    return out


def _quant_fp8(a, scale):
    import ml_dtypes
    return np.clip(a * scale, -FP8_MAX, FP8_MAX).astype(ml_dtypes.float8_e4m3)


def _pack_w13(a):
    """[D, 2H] -> [128, ND*2H], columns ordered (ht, w1|w3, dt, 128)."""
    r = a.reshape(ND, 128, 2, NH, 128)        # dt, p, w, ht, c
    r = r.transpose(1, 3, 2, 0, 4)            # p, ht, w, dt, c
    return np.ascontiguousarray(r.reshape(128, ND * H2))


def _pmajor(a, cols):
    """[rows=nd*128, cols] -> [128, nd*cols] partition-major layout."""
    nd = a.shape[0] // 128
    return np.ascontiguousarray(
        a.reshape(nd, 128, cols).transpose(1, 0, 2).reshape(128, nd * cols))


def kernel(hidden_states, gate_w, w1, w3, w2):
    from concourse import bass_utils

    hidden_states = np.asarray(hidden_states, dtype=np.float32)
    gate_w = np.asarray(gate_w, dtype=np.float32)
    w1 = np.asarray(w1, dtype=np.float32)
    w3 = np.asarray(w3, dtype=np.float32)
    w2 = np.asarray(w2, dtype=np.float32)

    x = hidden_states.reshape(T, D)
    idx, wts = _route(x, gate_w)

    tok_lists = []
    wt_lists = []
    for e in range(E):
        sel = np.nonzero(idx == e)
        tok_lists.append(sel[0])
        wt_lists.append(wts[sel])
    counts = np.array([len(t) for t in tok_lists])

    # pair largest with smallest; slot A = larger of the pair
    order = np.argsort(-counts, kind="stable")
    pairs = [(order[i], order[E - 1 - i]) for i in range(NCORES)]
    if any(counts[a] > SLOT_CAP[0] or counts[b] > SLOT_CAP[1]
           for a, b in pairs):
        return _moe_numpy(x, gate_w, w1, w3, w2).reshape(B, S, D)

    xq = _quant_fp8(x, SX)                                    # [T, D] fp8
    w13q = _quant_fp8(np.concatenate([w1, w3], axis=2), SW1)  # [E, D, 2H]
    w13q = np.stack([_pack_w13(w13q[e]) for e in range(E)])
    w2q = _quant_fp8(w2, SW2)
    w2q = np.stack([_pmajor(w2q[e], D) for e in range(E)])

    in_maps = []
    for c in range(NCORES):
        xg = np.zeros((TOT, D), dtype=xq.dtype)
        wgt = np.zeros(TOT, dtype=np.float32)
        for j, e in enumerate(pairs[c]):
            ne = counts[e]
            xg[SLOT_OFF[j]:SLOT_OFF[j] + ne] = xq[tok_lists[e]]
            wgt[SLOT_OFF[j]:SLOT_OFF[j] + ne] = wt_lists[e] / S2
        xgT = np.ascontiguousarray(xg.T)       # [D, TOT]
        ea, eb = pairs[c]
        m = {
            "w13": np.stack([w13q[ea], w13q[eb]]),
            "w2": np.stack([w2q[ea], w2q[eb]]),
            "wg": np.ascontiguousarray(wgt.reshape(NTT, 128).T),
        }
        for bi, (s, off, n) in enumerate(XBLOCKS):
            c0 = SLOT_OFF[s] + off
            m[f"xtb{bi}"] = _pmajor(xgT[:, c0:c0 + n], n)
        in_maps.append(m)

    res = bass_utils.run_bass_kernel_spmd(
        _program(), in_maps, core_ids=list(range(NCORES)))
    global _LAST_RESULTS
    _LAST_RESULTS = res

    out = x.copy()
    for c in range(NCORES):
        yc = np.asarray(res.results[c]["y"], dtype=np.float32)
        yc = yc.reshape(NTT * 128, D)
        for j, e in enumerate(pairs[c]):
            ne = counts[e]
            out[tok_lists[e]] += yc[SLOT_OFF[j]:SLOT_OFF[j] + ne]
    return out.reshape(B, S, D)


# revision 25
# speedup vs baseline: 1.1156x; 1.0090x over previous
"""Trainium2 Bass kernel: DeepSeek-style MoE layer (16 experts, top-2).

Strategy (expert-parallel, 8 cores):
  - Host computes the router (softmax + top-2 + renorm) in fp32 numpy and
    builds the token dispatch.  Experts are paired large-with-small onto
    cores; slot A holds up to 1152 tokens (9 tiles), slot B up to 1024
    (8 tiles).  Gathered tokens ship transposed ([D, slots]) in fp8-e4m3
    (scaled by SX), partition-major so every DMA row is one contiguous
    descriptor.
  - Device (per core, identical SPMD program), all matmuls fp8 DoubleRow
    (two 128-row contractions per instruction = 2x PE throughput):
        gT/uT = w13.T @ xT          (PSUM fp32 = S1 * true, [H_tile, tok])
        sg    = silu(gT / S1)       (scalar engine, bf16)
        hT    = sg * uT             (fp8, = S1 * h_true)
        y     = hT.T @ w2           (PSUM fp32 = S1*SW2 * true)
        ys    = y * wg'             (wg' = combine_weight / (S1*SW2))
    Stage-2 combine alternates vector/scalar engines to balance load.
  - Host scatter-adds the (already weighted) expert outputs into the
    residual stream.

Hardcoded for B=2, S=4096, D=1024, H=512, E=16, K=2.
"""

import numpy as np

B, S, D, H, E, TOPK = 2, 4096, 1024, 512, 16, 2
T = B * S
NCORES = 8
EPC = E // NCORES          # experts per core = 2
SLOT_CAP = [1152, 1024]    # token capacity per slot (A, B)
SLOT_OFF = [0, 1152]
TOT = sum(SLOT_CAP)        # 2176 token slots per core
NTT = TOT // 128           # 17 token tiles per core
# (slot, token offset within slot, length) — one xt DMA slab per entry.
# The small 128-token block runs LAST so the post-matmul tail drains only
# one tile's combine+DMA.
XBLOCKS = [(0, 0, 512), (0, 512, 512), (1, 0, 512),
           (1, 512, 512), (0, 1024, 128)]
ND = D // 128              # 8 d-tiles (stage-1 contraction)
NH = H // 128              # 4 h-tiles
H2 = 2 * H                 # w1|w3 fused column width

# fp8 scaling: x*SX and w13*SW1 keep operands inside e4m3's normal range
# (w has std 0.02, below e4m3's 2^-6 min normal unscaled).
SX = 2.0
SW1 = 8.0
SW2 = 8.0
S1 = SX * SW1              # scale of stage-1 PSUM (g, u)
S2 = S1 * SW2              # scale of stage-2 PSUM (y)
FP8_MAX = 240.0            # TRN e4m3 max normal

_PROG = None
_LAST_RESULTS = None


def _pair(ap):
    """[128, 2*c] AP -> [128, 2, c] view for DoubleRow matmul operands."""
    return ap.rearrange("p (two c) -> p two c", two=2)


def _build_program():
    import concourse.bacc as bacc
    import concourse.tile as tile
    from concourse import mybir

    BF = mybir.dt.bfloat16
    FP8 = mybir.dt.float8e4
    F32 = mybir.dt.float32
    AF = mybir.ActivationFunctionType
    DR = mybir.MatmulPerfMode.DoubleRow

    nc = bacc.Bacc("TRN2", target_bir_lowering=False, debug=False,
                   num_devices=NCORES)

    # DRAM I/O (per core), all partition-major: row p holds everything
    # partition p will need, contiguously.
    xt_ds = [nc.dram_tensor(f"xtb{bi}", [128, ND * n], FP8,
                            kind="ExternalInput")
             for bi, (_, _, n) in enumerate(XBLOCKS)]
    w13_d = nc.dram_tensor("w13", [EPC, 128, ND * H2], FP8,
                           kind="ExternalInput")
    w2_d = nc.dram_tensor("w2", [EPC, 128, NH * D], FP8, kind="ExternalInput")
    wg_d = nc.dram_tensor("wg", [128, NTT], F32, kind="ExternalInput")
    y_d = nc.dram_tensor("y", [NTT, 128, D], BF, kind="ExternalOutput")

    w13 = w13_d.ap()
    w2 = w2_d.ap()
    wg = wg_d.ap()
    y = y_d.ap()

    with tile.TileContext(nc) as tc:
        with (
            tc.tile_pool(name="wpool", bufs=1) as wpool,
            tc.tile_pool(name="hpool", bufs=2) as hpool,
            tc.tile_pool(name="ypool", bufs=6) as ypool,
            tc.tile_pool(name="ps1", bufs=2, space="PSUM") as ps1,
            tc.tile_pool(name="ps2", bufs=4, space="PSUM") as ps2,
        ):
            # ---- HAM warmup: dummy matmuls on a zeroed scratch tile so the
            # PE clock-gate opens while input DMAs stream in.
            warm = wpool.tile([128, 512], BF, tag="warm", name="warm")
            nc.vector.memset(warm[:], 0.0)
            wps = ps1.tile([128, 512], F32, tag="g", name="wps")
            for i in range(8):
                nc.tensor.matmul(wps[:], warm[:, 0:128], warm[:],
                                 start=(i == 0), stop=(i == 7))

            # ---- static SBUF-resident inputs ----
            wg_sb = wpool.tile([128, NTT], F32, tag="wg", name="wg")
            xt_sb = [wpool.tile([128, ND * n], FP8, tag=f"xtb{bi}",
                                name=f"xtb{bi}")
                     for bi, (_, _, n) in enumerate(XBLOCKS)]
            # free layout: (ht, w) major, then dt, then 128 cols.  One tile
            # per (expert, ht) chunk so a stage-1 group only waits for its
            # own ht slab, not the whole expert weight DMA.
            w13_sb = [[wpool.tile([128, 2 * ND * 128], FP8,
                                  tag=f"w13_{e}_{ht}", name=f"w13_{e}_{ht}")
                       for ht in range(NH)] for e in range(EPC)]
            w2_sb = [wpool.tile([128, NH * D], FP8, tag=f"w2_{e}",
                                name=f"w2_{e}") for e in range(EPC)]

            # DMA issue order = need order.  The first real matmul needs ALL
            # of xtb0 + w13[ht0]: dispatch those three slabs in parallel on
            # all three DMA-capable rings (sync, scalar, gpsimd), then stream
            # the rest on sync/scalar.  gpsimd afterwards carries wg + all y
            # output DMAs so input dispatch never queues behind outputs.
            HTC = 2 * ND * 128           # columns per ht-chunk of w13
            XH = ND * 512 // 2
            # Each DMA ring is a serial pipe (~100GB/s); transfers on a ring
            # complete in dispatch order.  Split every early tensor across
            # rings so the pieces move in parallel, ordered by need time.
            HHC = HTC // 2
            XT3 = (ND * 512) // 3 // 128 * 128   # xtb0 third, 128-aligned
            x0 = xt_ds[0].ap()
            # wave 1: w13[0][ht0] + xtb0 (first real matmul group)
            nc.sync.dma_start(w13_sb[0][0][:, 0:HHC], w13[0, :, 0:HHC])
            nc.scalar.dma_start(w13_sb[0][0][:, HHC:HTC], w13[0, :, HHC:HTC])
            nc.gpsimd.dma_start(xt_sb[0][:, 0:XT3], x0[:, 0:XT3])
            nc.sync.dma_start(xt_sb[0][:, XT3:2 * XT3], x0[:, XT3:2 * XT3])
            nc.scalar.dma_start(xt_sb[0][:, 2 * XT3:], x0[:, 2 * XT3:])
            # wave 2: remaining w13[0] ht slabs, split in halves
            for ht in range(1, NH):
                o = ht * HTC
                nc.sync.dma_start(w13_sb[0][ht][:, 0:HHC],
                                  w13[0, :, o:o + HHC])
                nc.scalar.dma_start(w13_sb[0][ht][:, HHC:HTC],
                                    w13[0, :, o + HHC:o + HTC])
            nc.gpsimd.dma_start(wg_sb[:], wg[:])
            # wave 3: next blocks' x, stage-2 weights, expert 1 weights
            nc.sync.dma_start(xt_sb[1][:, 0:XH], xt_ds[1].ap()[:, 0:XH])
            nc.scalar.dma_start(xt_sb[1][:, XH:], xt_ds[1].ap()[:, XH:])
            nc.sync.dma_start(w2_sb[0][:, 0:NH * D // 2],
                              w2[0, :, 0:NH * D // 2])
            nc.scalar.dma_start(w2_sb[0][:, NH * D // 2:],
                                w2[0, :, NH * D // 2:])
            nc.sync.dma_start(xt_sb[2][:, 0:XH], xt_ds[2].ap()[:, 0:XH])
            nc.scalar.dma_start(xt_sb[2][:, XH:], xt_ds[2].ap()[:, XH:])
            for ht in range(NH):
                o = ht * HTC
                ring = nc.gpsimd if ht < 2 else nc.scalar
                ring.dma_start(w13_sb[1][ht][:], w13[1, :, o:o + HTC])
            nc.sync.dma_start(xt_sb[3][:], xt_ds[3].ap()[:])
            nc.scalar.dma_start(w2_sb[1][:], w2[1])
            nc.gpsimd.dma_start(xt_sb[4][:], xt_ds[4].ap()[:])
            yring = [nc.sync, nc.gpsimd]

            # ---- compute ----
            for bi, (e, off, n) in enumerate(XBLOCKS):
                h_all = hpool.tile([128, NH * 512], FP8, tag="h", name="h")
                for ht in range(NH):
                    g = ps1.tile([128, 512], F32, tag="g", name="g")
                    u = ps1.tile([128, 512], F32, tag="u", name="u")
                    for w in range(2):
                        dst = g if w == 0 else u
                        for j in range(ND // 2):
                            o = (w * ND + 2 * j) * 128
                            nc.tensor.matmul(
                                dst[:, :n],
                                _pair(w13_sb[e][ht][:, o: o + 256]),
                                _pair(xt_sb[bi][:, 2 * j * n: (2 * j + 2) * n]),
                                start=(j == 0), stop=(j == ND // 2 - 1),
                                perf_mode=DR,
                            )
                    sg = hpool.tile([128, 512], BF, tag="sg", name="sg")
                    nc.scalar.activation(sg[:, :n], g[:, :n], AF.Silu,
                                         scale=1.0 / S1)
                    nc.vector.tensor_mul(h_all[:, ht * 512: ht * 512 + n],
                                         sg[:, :n], u[:, :n])
                for tt in range(n // 128):
                    gtt = (SLOT_OFF[e] + off) // 128 + tt
                    ys = ypool.tile([128, D], BF, tag="ys", name="ys")
                    for db in range(2):
                        yp = ps2.tile([128, 512], F32, tag="yp", name="yp")
                        for hp in range(NH // 2):
                            nc.tensor.matmul(
                                yp[:],
                                _pair(h_all[:, 2 * hp * 512:
                                            (2 * hp + 2) * 512])
                                [:, :, tt * 128:(tt + 1) * 128],
                                _pair(w2_sb[e][:, 2 * hp * D:
                                               (2 * hp + 2) * D])
                                [:, :, db * 512:(db + 1) * 512],
                                start=(hp == 0), stop=(hp == NH // 2 - 1),
                                perf_mode=DR,
                            )
                        if db == 0:
                            nc.vector.tensor_scalar_mul(
                                ys[:, 0:512], yp[:], wg_sb[:, gtt:gtt + 1])
                        else:
                            nc.scalar.mul(ys[:, 512:1024], yp[:],
                                          wg_sb[:, gtt:gtt + 1])
                    yring[gtt % 2].dma_start(y[gtt], ys[:])

    nc.compile()
    return nc


def _program():
    global _PROG
    if _PROG is None:
        _PROG = _build_program()
    return _PROG


def _route(x, gate_w):
    """fp32 softmax router + top-2 with renormalized weights (matches ref)."""
    logits = x @ gate_w.astype(np.float32)
    logits = logits - logits.max(axis=-1, keepdims=True)
    ex = np.exp(logits)
    scores = ex / ex.sum(axis=-1, keepdims=True)
    idx = np.argsort(-scores, axis=-1, kind="stable")[:, :TOPK]
    w = np.take_along_axis(scores, idx, axis=-1)
    w = w / w.sum(axis=-1, keepdims=True)
    return idx, w.astype(np.float32)


def _moe_numpy(x, gate_w, w1, w3, w2):
    """Slow exact fallback (only used if a capacity overflow ever happens)."""
    idx, wts = _route(x, gate_w)
    out = x.copy()
    for e in range(E):
        sel = np.nonzero(idx == e)
        toks = sel[0]
        ww = wts[sel]
        xe = x[toks]
        g = xe @ w1[e]
        u = xe @ w3[e]
        h = (g / (1.0 + np.exp(-g))) * u
        out[toks] += (h @ w2[e]) * ww[:, None]
    return out


def _quant_fp8(a, scale):
    import ml_dtypes
    return np.clip(a * scale, -FP8_MAX, FP8_MAX).astype(ml_dtypes.float8_e4m3)


def _pack_w13(a):
    """[D, 2H] -> [128, ND*2H], columns ordered (ht, w1|w3, dt, 128)."""
    r = a.reshape(ND, 128, 2, NH, 128)        # dt, p, w, ht, c
    r = r.transpose(1, 3, 2, 0, 4)            # p, ht, w, dt, c
    return np.ascontiguousarray(r.reshape(128, ND * H2))


def _pmajor(a, cols):
    """[rows=nd*128, cols] -> [128, nd*cols] partition-major layout."""
    nd = a.shape[0] // 128
    return np.ascontiguousarray(
        a.reshape(nd, 128, cols).transpose(1, 0, 2).reshape(128, nd * cols))


def kernel(hidden_states, gate_w, w1, w3, w2):
    from concourse import bass_utils

    hidden_states = np.asarray(hidden_states, dtype=np.float32)
    gate_w = np.asarray(gate_w, dtype=np.float32)
    w1 = np.asarray(w1, dtype=np.float32)
    w3 = np.asarray(w3, dtype=np.float32)
    w2 = np.asarray(w2, dtype=np.float32)

    x = hidden_states.reshape(T, D)
    idx, wts = _route(x, gate_w)

    tok_lists = []
    wt_lists = []
    for e in range(E):
        sel = np.nonzero(idx == e)
        tok_lists.append(sel[0])
        wt_lists.append(wts[sel])
    counts = np.array([len(t) for t in tok_lists])

    # pair largest with smallest; slot A = larger of the pair
    order = np.argsort(-counts, kind="stable")
    pairs = [(order[i], order[E - 1 - i]) for i in range(NCORES)]
    if any(counts[a] > SLOT_CAP[0] or counts[b] > SLOT_CAP[1]
           for a, b in pairs):
        return _moe_numpy(x, gate_w, w1, w3, w2).reshape(B, S, D)

    xq = _quant_fp8(x, SX)                                    # [T, D] fp8
    w13q = _quant_fp8(np.concatenate([w1, w3], axis=2), SW1)  # [E, D, 2H]
    w13q = np.stack([_pack_w13(w13q[e]) for e in range(E)])
    w2q = _quant_fp8(w2, SW2)
    w2q = np.stack([_pmajor(w2q[e], D) for e in range(E)])

    in_maps = []
    for c in range(NCORES):
        xg = np.zeros((TOT, D), dtype=xq.dtype)
        wgt = np.zeros(TOT, dtype=np.float32)
        for j, e in enumerate(pairs[c]):
            ne = counts[e]
            xg[SLOT_OFF[j]:SLOT_OFF[j] + ne] = xq[tok_lists[e]]
            wgt[SLOT_OFF[j]:SLOT_OFF[j] + ne] = wt_lists[e] / S2
        xgT = np.ascontiguousarray(xg.T)       # [D, TOT]
        ea, eb = pairs[c]
        m = {
            "w13": np.stack([w13q[ea], w13q[eb]]),
            "w2": np.stack([w2q[ea], w2q[eb]]),
            "wg": np.ascontiguousarray(wgt.reshape(NTT, 128).T),
        }
        for bi, (s, off, n) in enumerate(XBLOCKS):
            c0 = SLOT_OFF[s] + off
            m[f"xtb{bi}"] = _pmajor(xgT[:, c0:c0 + n], n)
        in_maps.append(m)

    res = bass_utils.run_bass_kernel_spmd(
        _program(), in_maps, core_ids=list(range(NCORES)))
    global _LAST_RESULTS
    _LAST_RESULTS = res

    out = x.copy()
    for c in range(NCORES):
        yc = np.asarray(res.results[c]["y"], dtype=np.float32)
        yc = yc.reshape(NTT * 128, D)
        for j, e in enumerate(pairs[c]):
            ne = counts[e]
            out[tok_lists[e]] += yc[SLOT_OFF[j]:SLOT_OFF[j] + ne]
    return out.reshape(B, S, D)


# revision 32
# speedup vs baseline: 1.1887x; 1.0655x over previous
"""Trainium2 Bass kernel: DeepSeek-style MoE layer (16 experts, top-2).

Strategy (expert-parallel, 8 cores):
  - Host computes the router (softmax + top-2 + renorm) in fp32 numpy and
    builds the token dispatch.  Experts are paired large-with-small onto
    cores; slot A holds up to 1152 tokens (9 tiles), slot B up to 1024
    (8 tiles).  Gathered tokens ship transposed ([D, slots]) in fp8-e4m3
    (scaled by SX), partition-major so every DMA row is one contiguous
    descriptor.
  - Device (per core, identical SPMD program), all matmuls fp8 DoubleRow
    (two 128-row contractions per instruction = 2x PE throughput):
        gT/uT = w13.T @ xT          (PSUM fp32 = S1 * true, [H_tile, tok])
        sg    = silu(gT / S1)       (scalar engine, bf16)
        hT    = sg * uT             (fp8, = S1 * h_true)
        y     = hT.T @ w2           (PSUM fp32 = S1*SW2 * true)
        ys    = y * wg'             (wg' = combine_weight / (S1*SW2))
    Stage-2 combine alternates vector/scalar engines to balance load.
  - Host scatter-adds the (already weighted) expert outputs into the
    residual stream.

Hardcoded for B=2, S=4096, D=1024, H=512, E=16, K=2.
"""

import numpy as np

B, S, D, H, E, TOPK = 2, 4096, 1024, 512, 16, 2
T = B * S
NCORES = 8
EPC = E // NCORES          # experts per core = 2
SLOT_CAP = [1152, 1024]    # token capacity per slot (A, B)
SLOT_OFF = [0, 1152]
TOT = sum(SLOT_CAP)        # 2176 token slots per core
NTT = TOT // 128           # 17 token tiles per core
# (slot, token offset within slot, length) — one xt DMA slab per entry.
# The small 128-token block runs LAST so the post-matmul tail drains only
# one tile's combine+DMA.
XBLOCKS = [(0, 0, 512), (0, 512, 512), (1, 0, 512),
           (1, 512, 512), (0, 1024, 128)]
ND = D // 128              # 8 d-tiles (stage-1 contraction)
NH = H // 128              # 4 h-tiles
H2 = 2 * H                 # w1|w3 fused column width

# fp8 scaling: x*SX and w13*SW1 keep operands inside e4m3's normal range
# (w has std 0.02, below e4m3's 2^-6 min normal unscaled).
SX = 2.0
SW1 = 8.0
SW2 = 8.0
S1 = SX * SW1              # scale of stage-1 PSUM (g, u)
S2 = S1 * SW2              # scale of stage-2 PSUM (y)
FP8_MAX = 240.0            # TRN e4m3 max normal

_PROG = None
_LAST_RESULTS = None


def _pair(ap):
    """[128, 2*c] AP -> [128, 2, c] view for DoubleRow matmul operands."""
    return ap.rearrange("p (two c) -> p two c", two=2)


def _build_program():
    import concourse.bacc as bacc
    import concourse.tile as tile
    from concourse import mybir

    BF = mybir.dt.bfloat16
    FP8 = mybir.dt.float8e4
    F32 = mybir.dt.float32
    AF = mybir.ActivationFunctionType
    DR = mybir.MatmulPerfMode.DoubleRow

    nc = bacc.Bacc("TRN2", target_bir_lowering=False, debug=False,
                   num_devices=NCORES)

    # DRAM I/O (per core), all partition-major: row p holds everything
    # partition p will need, contiguously.
    xt_ds = [nc.dram_tensor(f"xtb{bi}", [128, ND * n], FP8,
                            kind="ExternalInput")
             for bi, (_, _, n) in enumerate(XBLOCKS)]
    w13_d = nc.dram_tensor("w13", [EPC, 128, ND * H2], FP8,
                           kind="ExternalInput")
    # w2 packed as [e, db-half, 128, NH*512]: each half is its own SBUF
    # tile so stage-2 only waits for the half it reads.
    w2_d = nc.dram_tensor("w2", [EPC, 2, 128, NH * 512], FP8,
                          kind="ExternalInput")
    wg_d = nc.dram_tensor("wg", [128, NTT], F32, kind="ExternalInput")
    y_d = nc.dram_tensor("y", [NTT, 128, D], BF, kind="ExternalOutput")

    w13 = w13_d.ap()
    w2 = w2_d.ap()
    wg = wg_d.ap()
    y = y_d.ap()

    with tile.TileContext(nc) as tc:
        with (
            tc.tile_pool(name="wpool", bufs=1) as wpool,
            tc.tile_pool(name="hpool", bufs=2) as hpool,
            tc.tile_pool(name="ypool", bufs=6) as ypool,
            tc.tile_pool(name="ps1", bufs=2, space="PSUM") as ps1,
            tc.tile_pool(name="ps2", bufs=4, space="PSUM") as ps2,
        ):
            # ---- HAM warmup: dummy matmuls on a zeroed scratch tile so the
            # PE clock-gate opens while input DMAs stream in.
            warm = wpool.tile([128, 512], BF, tag="warm", name="warm")
            nc.vector.memset(warm[:], 0.0)
            wps = ps1.tile([128, 512], F32, tag="g", name="wps")
            for i in range(8):
                nc.tensor.matmul(wps[:], warm[:, 0:128], warm[:],
                                 start=(i == 0), stop=(i == 7))

            # ---- static SBUF-resident inputs ----
            wg_sb = wpool.tile([128, NTT], F32, tag="wg", name="wg")
            xt_sb = [wpool.tile([128, ND * n], FP8, tag=f"xtb{bi}",
                                name=f"xtb{bi}")
                     for bi, (_, _, n) in enumerate(XBLOCKS)]
            # free layout: (ht, w) major, then dt, then 128 cols.  One tile
            # per (expert, ht) chunk so a stage-1 group only waits for its
            # own ht slab, not the whole expert weight DMA.
            w13_sb = [[wpool.tile([128, 2 * ND * 128], FP8,
                                  tag=f"w13_{e}_{ht}", name=f"w13_{e}_{ht}")
                       for ht in range(NH)] for e in range(EPC)]
            w2_sb = [[wpool.tile([128, NH * 512], FP8, tag=f"w2_{e}_{db}",
                                 name=f"w2_{e}_{db}") for db in range(2)]
                     for e in range(EPC)]

            # DMA issue order = need order.  The first real matmul needs ALL
            # of xtb0 + w13[ht0]: dispatch those three slabs in parallel on
            # all three DMA-capable rings (sync, scalar, gpsimd), then stream
            # the rest on sync/scalar.  gpsimd afterwards carries wg + all y
            # output DMAs so input dispatch never queues behind outputs.
            HTC = 2 * ND * 128           # columns per ht-chunk of w13
            # Ring model (measured): each of the 3 DMA-capable rings (sync=SP,
            # scalar=ACT, gpsimd=Pool) is a serial pipe with a dispatch
            # credit of 4 — the 5th dma_start on a ring stalls that ENGINE's
            # queue until an older transfer completes.  The scalar engine
            # also runs silu, so it gets exactly 4 early DMAs and no more.
            # gpsimd carries all y outputs, so input DMAs never queue
            # behind output dispatches on sync.
            HHC = HTC // 2
            XT3 = (ND * 512) // 3 // 128 * 128   # xtb0 third, 128-aligned
            x0 = xt_ds[0].ap()
            # wave 1: w13[0][ht0] + xtb0 split 3 ways (first matmul group)
            nc.sync.dma_start(w13_sb[0][0][:, 0:HHC], w13[0, :, 0:HHC])
            nc.scalar.dma_start(w13_sb[0][0][:, HHC:HTC], w13[0, :, HHC:HTC])
            nc.gpsimd.dma_start(xt_sb[0][:, 0:XT3], x0[:, 0:XT3])
            nc.sync.dma_start(xt_sb[0][:, XT3:2 * XT3], x0[:, XT3:2 * XT3])
            nc.scalar.dma_start(xt_sb[0][:, 2 * XT3:], x0[:, 2 * XT3:])
            nc.gpsimd.dma_start(wg_sb[:], wg[:])
            # scalar's last two credit slots: stage-2 weights for expert 0
            nc.scalar.dma_start(w2_sb[0][0][:], w2[0, 0])
            nc.scalar.dma_start(w2_sb[0][1][:], w2[0, 1])
            # sync: remaining w13[0] slabs + x blocks in need order
            for ht in range(1, NH):
                nc.sync.dma_start(w13_sb[0][ht][:],
                                  w13[0, :, ht * HTC:(ht + 1) * HTC])
            nc.sync.dma_start(xt_sb[1][:], xt_ds[1].ap()[:])
            nc.sync.dma_start(xt_sb[2][:], xt_ds[2].ap()[:])
            nc.sync.dma_start(xt_sb[3][:], xt_ds[3].ap()[:])
            nc.sync.dma_start(w2_sb[1][0][:], w2[1, 0])
            nc.sync.dma_start(w2_sb[1][1][:], w2[1, 1])
            # gpsimd: expert-1 stage-1 weights + the small tail block
            for ht in range(NH):
                nc.gpsimd.dma_start(w13_sb[1][ht][:],
                                    w13[1, :, ht * HTC:(ht + 1) * HTC])
            nc.gpsimd.dma_start(xt_sb[4][:], xt_ds[4].ap()[:])

            # ---- compute ----
            for bi, (e, off, n) in enumerate(XBLOCKS):
                h_all = hpool.tile([128, NH * 512], FP8, tag="h", name="h")
                for ht in range(NH):
                    g = ps1.tile([128, 512], F32, tag="g", name="g")
                    u = ps1.tile([128, 512], F32, tag="u", name="u")
                    for w in range(2):
                        dst = g if w == 0 else u
                        for j in range(ND // 2):
                            o = (w * ND + 2 * j) * 128
                            nc.tensor.matmul(
                                dst[:, :n],
                                _pair(w13_sb[e][ht][:, o: o + 256]),
                                _pair(xt_sb[bi][:, 2 * j * n: (2 * j + 2) * n]),
                                start=(j == 0), stop=(j == ND // 2 - 1),
                                perf_mode=DR,
                            )
                    sg = hpool.tile([128, 512], BF, tag="sg", name="sg")
                    nc.scalar.activation(sg[:, :n], g[:, :n], AF.Silu,
                                         scale=1.0 / S1)
                    nc.vector.tensor_mul(h_all[:, ht * 512: ht * 512 + n],
                                         sg[:, :n], u[:, :n])
                for tt in range(n // 128):
                    gtt = (SLOT_OFF[e] + off) // 128 + tt
                    ys = ypool.tile([128, D], BF, tag="ys", name="ys")
                    for db in range(2):
                        yp = ps2.tile([128, 512], F32, tag="yp", name="yp")
                        for hp in range(NH // 2):
                            nc.tensor.matmul(
                                yp[:],
                                _pair(h_all[:, 2 * hp * 512:
                                            (2 * hp + 2) * 512])
                                [:, :, tt * 128:(tt + 1) * 128],
                                _pair(w2_sb[e][db][:, 2 * hp * 512:
                                                   (2 * hp + 2) * 512]),
                                start=(hp == 0), stop=(hp == NH // 2 - 1),
                                perf_mode=DR,
                            )
                        if db == 0:
                            nc.vector.tensor_scalar_mul(
                                ys[:, 0:512], yp[:], wg_sb[:, gtt:gtt + 1])
                        else:
                            nc.scalar.mul(ys[:, 512:1024], yp[:],
                                          wg_sb[:, gtt:gtt + 1])
                    nc.gpsimd.dma_start(y[gtt], ys[:])

    nc.compile()
    return nc


def _program():
    global _PROG
    if _PROG is None:
        _PROG = _build_program()
    return _PROG


def _route(x, gate_w):
    """fp32 softmax router + top-2 with renormalized weights (matches ref)."""
    logits = x @ gate_w.astype(np.float32)
    logits = logits - logits.max(axis=-1, keepdims=True)
    ex = np.exp(logits)
    scores = ex / ex.sum(axis=-1, keepdims=True)
    idx = np.argsort(-scores, axis=-1, kind="stable")[:, :TOPK]
    w = np.take_along_axis(scores, idx, axis=-1)
    w = w / w.sum(axis=-1, keepdims=True)
    return idx, w.astype(np.float32)


def _moe_numpy(x, gate_w, w1, w3, w2):
    """Slow exact fallback (only used if a capacity overflow ever happens)."""
    idx, wts = _route(x, gate_w)
    out = x.copy()
    for e in range(E):
        sel = np.nonzero(idx == e)
        toks = sel[0]
        ww = wts[sel]
        xe = x[toks]
        g = xe @ w1[e]
        u = xe @ w3[e]
        h = (g / (1.0 + np.exp(-g))) * u
        out[toks] += (h @ w2[e]) * ww[:, None]
    return out


def _quant_fp8(a, scale):
    import ml_dtypes
    return np.clip(a * scale, -FP8_MAX, FP8_MAX).astype(ml_dtypes.float8_e4m3)


def _pack_w13(a):
    """[D, 2H] -> [128, ND*2H], columns ordered (ht, w1|w3, dt, 128)."""
    r = a.reshape(ND, 128, 2, NH, 128)        # dt, p, w, ht, c
    r = r.transpose(1, 3, 2, 0, 4)            # p, ht, w, dt, c
    return np.ascontiguousarray(r.reshape(128, ND * H2))


def _pmajor(a, cols):
    """[rows=nd*128, cols] -> [128, nd*cols] partition-major layout."""
    nd = a.shape[0] // 128
    return np.ascontiguousarray(
        a.reshape(nd, 128, cols).transpose(1, 0, 2).reshape(128, nd * cols))


def kernel(hidden_states, gate_w, w1, w3, w2):
    from concourse import bass_utils

    hidden_states = np.asarray(hidden_states, dtype=np.float32)
    gate_w = np.asarray(gate_w, dtype=np.float32)
    w1 = np.asarray(w1, dtype=np.float32)
    w3 = np.asarray(w3, dtype=np.float32)
    w2 = np.asarray(w2, dtype=np.float32)

    x = hidden_states.reshape(T, D)
    idx, wts = _route(x, gate_w)

    tok_lists = []
    wt_lists = []
    for e in range(E):
        sel = np.nonzero(idx == e)
        tok_lists.append(sel[0])
        wt_lists.append(wts[sel])
    counts = np.array([len(t) for t in tok_lists])

    # pair largest with smallest; slot A = larger of the pair
    order = np.argsort(-counts, kind="stable")
    pairs = [(order[i], order[E - 1 - i]) for i in range(NCORES)]
    if any(counts[a] > SLOT_CAP[0] or counts[b] > SLOT_CAP[1]
           for a, b in pairs):
        return _moe_numpy(x, gate_w, w1, w3, w2).reshape(B, S, D)

    xq = _quant_fp8(x, SX)                                    # [T, D] fp8
    w13q = _quant_fp8(np.concatenate([w1, w3], axis=2), SW1)  # [E, D, 2H]
    w13q = np.stack([_pack_w13(w13q[e]) for e in range(E)])
    w2q = _quant_fp8(w2, SW2)
    # [128, NH*D] (ht,d)-major -> [2, 128, NH*512] db-half-major
    w2q = np.stack([
        np.ascontiguousarray(
            _pmajor(w2q[e], D).reshape(128, NH, 2, 512)
            .transpose(2, 0, 1, 3).reshape(2, 128, NH * 512))
        for e in range(E)])

    in_maps = []
    for c in range(NCORES):
        xg = np.zeros((TOT, D), dtype=xq.dtype)
        wgt = np.zeros(TOT, dtype=np.float32)
        for j, e in enumerate(pairs[c]):
            ne = counts[e]
            xg[SLOT_OFF[j]:SLOT_OFF[j] + ne] = xq[tok_lists[e]]
            wgt[SLOT_OFF[j]:SLOT_OFF[j] + ne] = wt_lists[e] / S2
        xgT = np.ascontiguousarray(xg.T)       # [D, TOT]
        ea, eb = pairs[c]
        m = {
            "w13": np.stack([w13q[ea], w13q[eb]]),
            "w2": np.stack([w2q[ea], w2q[eb]]),
            "wg": np.ascontiguousarray(wgt.reshape(NTT, 128).T),
        }
        for bi, (s, off, n) in enumerate(XBLOCKS):
            c0 = SLOT_OFF[s] + off
            m[f"xtb{bi}"] = _pmajor(xgT[:, c0:c0 + n], n)
        in_maps.append(m)

    res = bass_utils.run_bass_kernel_spmd(
        _program(), in_maps, core_ids=list(range(NCORES)))
    global _LAST_RESULTS
    _LAST_RESULTS = res

    out = x.copy()
    for c in range(NCORES):
        yc = np.asarray(res.results[c]["y"], dtype=np.float32)
        yc = yc.reshape(NTT * 128, D)
        for j, e in enumerate(pairs[c]):
            ne = counts[e]
            out[tok_lists[e]] += yc[SLOT_OFF[j]:SLOT_OFF[j] + ne]
    return out.reshape(B, S, D)


# revision 34
# speedup vs baseline: 1.2262x; 1.0315x over previous
"""Trainium2 Bass kernel: DeepSeek-style MoE layer (16 experts, top-2).

Strategy (expert-parallel, 8 cores):
  - Host computes the router (softmax + top-2 + renorm) in fp32 numpy and
    builds the token dispatch.  Experts are paired large-with-small onto
    cores; slot A holds up to 1152 tokens (9 tiles), slot B up to 1024
    (8 tiles).  Gathered tokens ship transposed ([D, slots]) in fp8-e4m3
    (scaled by SX), partition-major so every DMA row is one contiguous
    descriptor.
  - Device (per core, identical SPMD program), all matmuls fp8 DoubleRow
    (two 128-row contractions per instruction = 2x PE throughput):
        gT/uT = w13.T @ xT          (PSUM fp32 = S1 * true, [H_tile, tok])
        sg    = silu(gT / S1)       (scalar engine, bf16)
        hT    = sg * uT             (fp8, = S1 * h_true)
        y     = hT.T @ w2           (PSUM fp32 = S1*SW2 * true)
        ys    = y * wg'             (wg' = combine_weight / (S1*SW2))
    Stage-2 combine alternates vector/scalar engines to balance load.
  - Host scatter-adds the (already weighted) expert outputs into the
    residual stream.

Hardcoded for B=2, S=4096, D=1024, H=512, E=16, K=2.
"""

import numpy as np

B, S, D, H, E, TOPK = 2, 4096, 1024, 512, 16, 2
T = B * S
NCORES = 8
EPC = E // NCORES          # experts per core = 2
SLOT_CAP = [1152, 1024]    # token capacity per slot (A, B)
SLOT_OFF = [0, 1152]
TOT = sum(SLOT_CAP)        # 2176 token slots per core
NTT = TOT // 128           # 17 token tiles per core
# (slot, token offset within slot, length) — one xt DMA slab per entry.
# The small 128-token block runs LAST so the post-matmul tail drains only
# one tile's combine+DMA.
XBLOCKS = [(0, 0, 512), (0, 512, 512), (1, 0, 512),
           (1, 512, 512), (0, 1024, 128)]
ND = D // 128              # 8 d-tiles (stage-1 contraction)
NH = H // 128              # 4 h-tiles
H2 = 2 * H                 # w1|w3 fused column width

# fp8 scaling: x*SX and w13*SW1 keep operands inside e4m3's normal range
# (w has std 0.02, below e4m3's 2^-6 min normal unscaled).
SX = 2.0
SW1 = 8.0
SW2 = 8.0
S1 = SX * SW1              # scale of stage-1 PSUM (g, u)
S2 = S1 * SW2              # scale of stage-2 PSUM (y)
FP8_MAX = 240.0            # TRN e4m3 max normal

_PROG = None
_LAST_RESULTS = None


def _pair(ap):
    """[128, 2*c] AP -> [128, 2, c] view for DoubleRow matmul operands."""
    return ap.rearrange("p (two c) -> p two c", two=2)


def _build_program():
    import concourse.bacc as bacc
    import concourse.tile as tile
    from concourse import mybir

    BF = mybir.dt.bfloat16
    FP8 = mybir.dt.float8e4
    F32 = mybir.dt.float32
    AF = mybir.ActivationFunctionType
    DR = mybir.MatmulPerfMode.DoubleRow

    nc = bacc.Bacc("TRN2", target_bir_lowering=False, debug=False,
                   num_devices=NCORES)

    # DRAM I/O (per core), all partition-major: row p holds everything
    # partition p will need, contiguously.
    xt_ds = [nc.dram_tensor(f"xtb{bi}", [128, ND * n], FP8,
                            kind="ExternalInput")
             for bi, (_, _, n) in enumerate(XBLOCKS)]
    w13_d = nc.dram_tensor("w13", [EPC, 128, ND * H2], FP8,
                           kind="ExternalInput")
    # w2 packed as [e, db-half, 128, NH*512]: each half is its own SBUF
    # tile so stage-2 only waits for the half it reads.
    w2_d = nc.dram_tensor("w2", [EPC, 2, 128, NH * 512], FP8,
                          kind="ExternalInput")
    wg_d = nc.dram_tensor("wg", [128, NTT], F32, kind="ExternalInput")
    y_d = nc.dram_tensor("y", [NTT, 128, D], BF, kind="ExternalOutput")

    w13 = w13_d.ap()
    w2 = w2_d.ap()
    wg = wg_d.ap()
    y = y_d.ap()

    with tile.TileContext(nc) as tc:
        with (
            tc.tile_pool(name="wpool", bufs=1) as wpool,
            tc.tile_pool(name="hpool", bufs=2) as hpool,
            tc.tile_pool(name="ypool", bufs=6) as ypool,
            tc.tile_pool(name="ps1", bufs=2, space="PSUM") as ps1,
            tc.tile_pool(name="ps2", bufs=4, space="PSUM") as ps2,
        ):
            # ---- HAM warmup: dummy matmuls on a zeroed scratch tile so the
            # PE clock-gate opens while input DMAs stream in.
            warm = wpool.tile([128, 512], BF, tag="warm", name="warm")
            nc.vector.memset(warm[:], 0.0)
            wps = ps1.tile([128, 512], F32, tag="g", name="wps")
            for i in range(8):
                nc.tensor.matmul(wps[:], warm[:, 0:128], warm[:],
                                 start=(i == 0), stop=(i == 7))

            # ---- static SBUF-resident inputs ----
            wg_sb = wpool.tile([128, NTT], F32, tag="wg", name="wg")
            xt_sb = [wpool.tile([128, ND * n], FP8, tag=f"xtb{bi}",
                                name=f"xtb{bi}")
                     for bi, (_, _, n) in enumerate(XBLOCKS)]
            # free layout: (ht, w) major, then dt, then 128 cols.  One tile
            # per (expert, ht) chunk so a stage-1 group only waits for its
            # own ht slab, not the whole expert weight DMA.
            w13_sb = [[wpool.tile([128, 2 * ND * 128], FP8,
                                  tag=f"w13_{e}_{ht}", name=f"w13_{e}_{ht}")
                       for ht in range(NH)] for e in range(EPC)]
            w2_sb = [[wpool.tile([128, NH * 512], FP8, tag=f"w2_{e}_{db}",
                                 name=f"w2_{e}_{db}") for db in range(2)]
                     for e in range(EPC)]

            # DMA issue order = need order.  The first real matmul needs ALL
            # of xtb0 + w13[ht0]: dispatch those three slabs in parallel on
            # all three DMA-capable rings (sync, scalar, gpsimd), then stream
            # the rest on sync/scalar.  gpsimd afterwards carries wg + all y
            # output DMAs so input dispatch never queues behind outputs.
            HTC = 2 * ND * 128           # columns per ht-chunk of w13
            # Ring model (measured): each of the 3 DMA-capable rings (sync=SP,
            # scalar=ACT, gpsimd=Pool) is a serial pipe with a dispatch
            # credit of 4 — the 5th dma_start on a ring stalls that ENGINE's
            # queue until an older transfer completes.  The scalar engine
            # also runs silu, so it gets exactly 4 early DMAs and no more.
            # gpsimd carries all y outputs, so input DMAs never queue
            # behind output dispatches on sync.
            HHC = HTC // 2
            XT3 = (ND * 512) // 3 // 128 * 128   # xtb0 third, 128-aligned
            x0 = xt_ds[0].ap()
            # wave 1: w13[0][ht0] + xtb0 split 3 ways (first matmul group)
            nc.sync.dma_start(w13_sb[0][0][:, 0:HHC], w13[0, :, 0:HHC])
            nc.scalar.dma_start(w13_sb[0][0][:, HHC:HTC], w13[0, :, HHC:HTC])
            nc.gpsimd.dma_start(xt_sb[0][:, 0:XT3], x0[:, 0:XT3])
            nc.sync.dma_start(xt_sb[0][:, XT3:2 * XT3], x0[:, XT3:2 * XT3])
            nc.scalar.dma_start(xt_sb[0][:, 2 * XT3:], x0[:, 2 * XT3:])
            nc.gpsimd.dma_start(wg_sb[:], wg[:])
            # scalar's last two credit slots: stage-2 weights for expert 0
            nc.scalar.dma_start(w2_sb[0][0][:], w2[0, 0])
            nc.scalar.dma_start(w2_sb[0][1][:], w2[0, 1])
            # remaining w13[0] slabs: ht1/ht3 on sync, ht2 on gpsimd
            nc.sync.dma_start(w13_sb[0][1][:], w13[0, :, HTC:2 * HTC])
            nc.gpsimd.dma_start(w13_sb[0][2][:], w13[0, :, 2 * HTC:3 * HTC])
            nc.sync.dma_start(w13_sb[0][3][:], w13[0, :, 3 * HTC:4 * HTC])
            nc.sync.dma_start(xt_sb[1][:], xt_ds[1].ap()[:])
            nc.sync.dma_start(xt_sb[2][:], xt_ds[2].ap()[:])
            nc.sync.dma_start(xt_sb[3][:], xt_ds[3].ap()[:])
            nc.sync.dma_start(w2_sb[1][0][:], w2[1, 0])
            nc.sync.dma_start(w2_sb[1][1][:], w2[1, 1])
            # gpsimd: expert-1 stage-1 weights + the small tail block
            for ht in range(NH):
                nc.gpsimd.dma_start(w13_sb[1][ht][:],
                                    w13[1, :, ht * HTC:(ht + 1) * HTC])
            nc.gpsimd.dma_start(xt_sb[4][:], xt_ds[4].ap()[:])

            # ---- compute ----
            for bi, (e, off, n) in enumerate(XBLOCKS):
                h_all = hpool.tile([128, NH * 512], FP8, tag="h", name="h")
                for ht in range(NH):
                    g = ps1.tile([128, 512], F32, tag="g", name="g")
                    u = ps1.tile([128, 512], F32, tag="u", name="u")
                    for w in range(2):
                        dst = g if w == 0 else u
                        for j in range(ND // 2):
                            o = (w * ND + 2 * j) * 128
                            nc.tensor.matmul(
                                dst[:, :n],
                                _pair(w13_sb[e][ht][:, o: o + 256]),
                                _pair(xt_sb[bi][:, 2 * j * n: (2 * j + 2) * n]),
                                start=(j == 0), stop=(j == ND // 2 - 1),
                                perf_mode=DR,
                            )
                    sg = hpool.tile([128, 512], BF, tag="sg", name="sg")
                    nc.scalar.activation(sg[:, :n], g[:, :n], AF.Silu,
                                         scale=1.0 / S1)
                    nc.vector.tensor_mul(h_all[:, ht * 512: ht * 512 + n],
                                         sg[:, :n], u[:, :n])
                for tt in range(n // 128):
                    gtt = (SLOT_OFF[e] + off) // 128 + tt
                    ys = ypool.tile([128, D], BF, tag="ys", name="ys")
                    for db in range(2):
                        yp = ps2.tile([128, 512], F32, tag="yp", name="yp")
                        for hp in range(NH // 2):
                            nc.tensor.matmul(
                                yp[:],
                                _pair(h_all[:, 2 * hp * 512:
                                            (2 * hp + 2) * 512])
                                [:, :, tt * 128:(tt + 1) * 128],
                                _pair(w2_sb[e][db][:, 2 * hp * 512:
                                                   (2 * hp + 2) * 512]),
                                start=(hp == 0), stop=(hp == NH // 2 - 1),
                                perf_mode=DR,
                            )
                        if db == 0:
                            nc.vector.tensor_scalar_mul(
                                ys[:, 0:512], yp[:], wg_sb[:, gtt:gtt + 1])
                        else:
                            nc.scalar.mul(ys[:, 512:1024], yp[:],
                                          wg_sb[:, gtt:gtt + 1])
                    if bi == len(XBLOCKS) - 1:
                        # final tile: split across two idle rings to shorten
                        # the drain tail
                        nc.sync.dma_start(y[gtt, :, 0:512], ys[:, 0:512])
                        nc.scalar.dma_start(y[gtt, :, 512:1024],
                                            ys[:, 512:1024])
                    elif bi == len(XBLOCKS) - 2:
                        # tiles written late go on the by-then-idle sync ring
                        nc.sync.dma_start(y[gtt], ys[:])
                    else:
                        nc.gpsimd.dma_start(y[gtt], ys[:])

    nc.compile()
    return nc


def _program():
    global _PROG
    if _PROG is None:
        _PROG = _build_program()
    return _PROG


def _route(x, gate_w):
    """fp32 softmax router + top-2 with renormalized weights (matches ref)."""
    logits = x @ gate_w.astype(np.float32)
    logits = logits - logits.max(axis=-1, keepdims=True)
    ex = np.exp(logits)
    scores = ex / ex.sum(axis=-1, keepdims=True)
    idx = np.argsort(-scores, axis=-1, kind="stable")[:, :TOPK]
    w = np.take_along_axis(scores, idx, axis=-1)
    w = w / w.sum(axis=-1, keepdims=True)
    return idx, w.astype(np.float32)


def _moe_numpy(x, gate_w, w1, w3, w2):
    """Slow exact fallback (only used if a capacity overflow ever happens)."""
    idx, wts = _route(x, gate_w)
    out = x.copy()
    for e in range(E):
        sel = np.nonzero(idx == e)
        toks = sel[0]
        ww = wts[sel]
        xe = x[toks]
        g = xe @ w1[e]
        u = xe @ w3[e]
        h = (g / (1.0 + np.exp(-g))) * u
        out[toks] += (h @ w2[e]) * ww[:, None]
    return out


def _quant_fp8(a, scale):
    import ml_dtypes
    return np.clip(a * scale, -FP8_MAX, FP8_MAX).astype(ml_dtypes.float8_e4m3)


def _pack_w13(a):
    """[D, 2H] -> [128, ND*2H], columns ordered (ht, w1|w3, dt, 128)."""
    r = a.reshape(ND, 128, 2, NH, 128)        # dt, p, w, ht, c
    r = r.transpose(1, 3, 2, 0, 4)            # p, ht, w, dt, c
    return np.ascontiguousarray(r.reshape(128, ND * H2))


def _pmajor(a, cols):
    """[rows=nd*128, cols] -> [128, nd*cols] partition-major layout."""
    nd = a.shape[0] // 128
    return np.ascontiguousarray(
        a.reshape(nd, 128, cols).transpose(1, 0, 2).reshape(128, nd * cols))


def kernel(hidden_states, gate_w, w1, w3, w2):
    from concourse import bass_utils

    hidden_states = np.asarray(hidden_states, dtype=np.float32)
    gate_w = np.asarray(gate_w, dtype=np.float32)
    w1 = np.asarray(w1, dtype=np.float32)
    w3 = np.asarray(w3, dtype=np.float32)
    w2 = np.asarray(w2, dtype=np.float32)

    x = hidden_states.reshape(T, D)
    idx, wts = _route(x, gate_w)

    tok_lists = []
    wt_lists = []
    for e in range(E):
        sel = np.nonzero(idx == e)
        tok_lists.append(sel[0])
        wt_lists.append(wts[sel])
    counts = np.array([len(t) for t in tok_lists])

    # pair largest with smallest; slot A = larger of the pair
    order = np.argsort(-counts, kind="stable")
    pairs = [(order[i], order[E - 1 - i]) for i in range(NCORES)]
    if any(counts[a] > SLOT_CAP[0] or counts[b] > SLOT_CAP[1]
           for a, b in pairs):
        return _moe_numpy(x, gate_w, w1, w3, w2).reshape(B, S, D)

    xq = _quant_fp8(x, SX)                                    # [T, D] fp8
    w13q = _quant_fp8(np.concatenate([w1, w3], axis=2), SW1)  # [E, D, 2H]
    w13q = np.stack([_pack_w13(w13q[e]) for e in range(E)])
    w2q = _quant_fp8(w2, SW2)
    # [128, NH*D] (ht,d)-major -> [2, 128, NH*512] db-half-major
    w2q = np.stack([
        np.ascontiguousarray(
            _pmajor(w2q[e], D).reshape(128, NH, 2, 512)
            .transpose(2, 0, 1, 3).reshape(2, 128, NH * 512))
        for e in range(E)])

    in_maps = []
    for c in range(NCORES):
        xg = np.zeros((TOT, D), dtype=xq.dtype)
        wgt = np.zeros(TOT, dtype=np.float32)
        for j, e in enumerate(pairs[c]):
            ne = counts[e]
            xg[SLOT_OFF[j]:SLOT_OFF[j] + ne] = xq[tok_lists[e]]
            wgt[SLOT_OFF[j]:SLOT_OFF[j] + ne] = wt_lists[e] / S2
        xgT = np.ascontiguousarray(xg.T)       # [D, TOT]
        ea, eb = pairs[c]
        m = {
            "w13": np.stack([w13q[ea], w13q[eb]]),
            "w2": np.stack([w2q[ea], w2q[eb]]),
            "wg": np.ascontiguousarray(wgt.reshape(NTT, 128).T),
        }
        for bi, (s, off, n) in enumerate(XBLOCKS):
            c0 = SLOT_OFF[s] + off
            m[f"xtb{bi}"] = _pmajor(xgT[:, c0:c0 + n], n)
        in_maps.append(m)

    res = bass_utils.run_bass_kernel_spmd(
        _program(), in_maps, core_ids=list(range(NCORES)))
    global _LAST_RESULTS
    _LAST_RESULTS = res

    out = x.copy()
    for c in range(NCORES):
        yc = np.asarray(res.results[c]["y"], dtype=np.float32)
        yc = yc.reshape(NTT * 128, D)
        for j, e in enumerate(pairs[c]):
            ne = counts[e]
            out[tok_lists[e]] += yc[SLOT_OFF[j]:SLOT_OFF[j] + ne]
    return out.reshape(B, S, D)


# revision 35
# speedup vs baseline: 1.2408x; 1.0119x over previous
"""Trainium2 Bass kernel: DeepSeek-style MoE layer (16 experts, top-2).

Strategy (expert-parallel, 8 cores):
  - Host computes the router (softmax + top-2 + renorm) in fp32 numpy and
    builds the token dispatch.  Experts are paired large-with-small onto
    cores; slot A holds up to 1152 tokens (9 tiles), slot B up to 1024
    (8 tiles).  Gathered tokens ship transposed ([D, slots]) in fp8-e4m3
    (scaled by SX), partition-major so every DMA row is one contiguous
    descriptor.
  - Device (per core, identical SPMD program), all matmuls fp8 DoubleRow
    (two 128-row contractions per instruction = 2x PE throughput):
        gT/uT = w13.T @ xT          (PSUM fp32 = S1 * true, [H_tile, tok])
        sg    = silu(gT / S1)       (scalar engine, bf16)
        hT    = sg * uT             (fp8, = S1 * h_true)
        y     = hT.T @ w2           (PSUM fp32 = S1*SW2 * true)
        ys    = y * wg'             (wg' = combine_weight / (S1*SW2))
    Stage-2 combine alternates vector/scalar engines to balance load.
  - Host scatter-adds the (already weighted) expert outputs into the
    residual stream.

Hardcoded for B=2, S=4096, D=1024, H=512, E=16, K=2.
"""

import numpy as np

B, S, D, H, E, TOPK = 2, 4096, 1024, 512, 16, 2
T = B * S
NCORES = 8
EPC = E // NCORES          # experts per core = 2
SLOT_CAP = [1152, 1024]    # token capacity per slot (A, B)
SLOT_OFF = [0, 1152]
TOT = sum(SLOT_CAP)        # 2176 token slots per core
NTT = TOT // 128           # 17 token tiles per core
# (slot, token offset within slot, length) — one xt DMA slab per entry.
# The small 128-token block runs LAST so the post-matmul tail drains only
# one tile's combine+DMA.
XBLOCKS = [(0, 0, 512), (0, 512, 512), (1, 0, 512),
           (1, 512, 512), (0, 1024, 128)]
ND = D // 128              # 8 d-tiles (stage-1 contraction)
NH = H // 128              # 4 h-tiles
H2 = 2 * H                 # w1|w3 fused column width

# fp8 scaling: x*SX and w13*SW1 keep operands inside e4m3's normal range
# (w has std 0.02, below e4m3's 2^-6 min normal unscaled).
SX = 2.0
SW1 = 8.0
SW2 = 8.0
S1 = SX * SW1              # scale of stage-1 PSUM (g, u)
S2 = S1 * SW2              # scale of stage-2 PSUM (y)
FP8_MAX = 240.0            # TRN e4m3 max normal

_PROG = None
_LAST_RESULTS = None


def _pair(ap):
    """[128, 2*c] AP -> [128, 2, c] view for DoubleRow matmul operands."""
    return ap.rearrange("p (two c) -> p two c", two=2)


def _build_program():
    import concourse.bacc as bacc
    import concourse.tile as tile
    from concourse import mybir

    BF = mybir.dt.bfloat16
    FP8 = mybir.dt.float8e4
    F32 = mybir.dt.float32
    AF = mybir.ActivationFunctionType
    DR = mybir.MatmulPerfMode.DoubleRow

    nc = bacc.Bacc("TRN2", target_bir_lowering=False, debug=False,
                   num_devices=NCORES)

    # DRAM I/O (per core), all partition-major: row p holds everything
    # partition p will need, contiguously.
    xt_ds = [nc.dram_tensor(f"xtb{bi}", [128, ND * n], FP8,
                            kind="ExternalInput")
             for bi, (_, _, n) in enumerate(XBLOCKS)]
    w13_d = nc.dram_tensor("w13", [EPC, 128, ND * H2], FP8,
                           kind="ExternalInput")
    # w2 packed as [e, db-half, 128, NH*512]: each half is its own SBUF
    # tile so stage-2 only waits for the half it reads.
    w2_d = nc.dram_tensor("w2", [EPC, 2, 128, NH * 512], FP8,
                          kind="ExternalInput")
    wg_d = nc.dram_tensor("wg", [128, NTT], F32, kind="ExternalInput")
    y_d = nc.dram_tensor("y", [NTT, 128, D], BF, kind="ExternalOutput")

    w13 = w13_d.ap()
    w2 = w2_d.ap()
    wg = wg_d.ap()
    y = y_d.ap()

    with tile.TileContext(nc) as tc:
        with (
            tc.tile_pool(name="wpool", bufs=1) as wpool,
            tc.tile_pool(name="hpool", bufs=2) as hpool,
            tc.tile_pool(name="ypool", bufs=6) as ypool,
            tc.tile_pool(name="ps1", bufs=2, space="PSUM") as ps1,
            tc.tile_pool(name="ps2", bufs=4, space="PSUM") as ps2,
        ):
            # ---- HAM warmup: dummy matmuls on a zeroed scratch tile so the
            # PE clock-gate opens while input DMAs stream in.
            warm = wpool.tile([128, 512], BF, tag="warm", name="warm")
            nc.vector.memset(warm[:], 0.0)
            wps = ps1.tile([128, 512], F32, tag="g", name="wps")
            for i in range(11):
                nc.tensor.matmul(wps[:], warm[:, 0:128], warm[:],
                                 start=(i == 0), stop=(i == 10))

            # ---- static SBUF-resident inputs ----
            wg_sb = wpool.tile([128, NTT], F32, tag="wg", name="wg")
            xt_sb = [wpool.tile([128, ND * n], FP8, tag=f"xtb{bi}",
                                name=f"xtb{bi}")
                     for bi, (_, _, n) in enumerate(XBLOCKS)]
            # free layout: (ht, w) major, then dt, then 128 cols.  One tile
            # per (expert, ht) chunk so a stage-1 group only waits for its
            # own ht slab, not the whole expert weight DMA.
            w13_sb = [[wpool.tile([128, 2 * ND * 128], FP8,
                                  tag=f"w13_{e}_{ht}", name=f"w13_{e}_{ht}")
                       for ht in range(NH)] for e in range(EPC)]
            w2_sb = [[wpool.tile([128, NH * 512], FP8, tag=f"w2_{e}_{db}",
                                 name=f"w2_{e}_{db}") for db in range(2)]
                     for e in range(EPC)]

            # DMA issue order = need order.  The first real matmul needs ALL
            # of xtb0 + w13[ht0]: dispatch those three slabs in parallel on
            # all three DMA-capable rings (sync, scalar, gpsimd), then stream
            # the rest on sync/scalar.  gpsimd afterwards carries wg + all y
            # output DMAs so input dispatch never queues behind outputs.
            HTC = 2 * ND * 128           # columns per ht-chunk of w13
            # Ring model (measured): each of the 3 DMA-capable rings (sync=SP,
            # scalar=ACT, gpsimd=Pool) is a serial pipe with a dispatch
            # credit of 4 — the 5th dma_start on a ring stalls that ENGINE's
            # queue until an older transfer completes.  The scalar engine
            # also runs silu, so it gets exactly 4 early DMAs and no more.
            # gpsimd carries all y outputs, so input DMAs never queue
            # behind output dispatches on sync.
            HHC = HTC // 2
            XT3 = (ND * 512) // 3 // 128 * 128   # xtb0 third, 128-aligned
            x0 = xt_ds[0].ap()
            # wave 1: w13[0][ht0] + xtb0 split 3 ways (first matmul group)
            nc.sync.dma_start(w13_sb[0][0][:, 0:HHC], w13[0, :, 0:HHC])
            nc.scalar.dma_start(w13_sb[0][0][:, HHC:HTC], w13[0, :, HHC:HTC])
            nc.gpsimd.dma_start(xt_sb[0][:, 0:XT3], x0[:, 0:XT3])
            nc.sync.dma_start(xt_sb[0][:, XT3:2 * XT3], x0[:, XT3:2 * XT3])
            nc.scalar.dma_start(xt_sb[0][:, 2 * XT3:], x0[:, 2 * XT3:])
            nc.gpsimd.dma_start(wg_sb[:], wg[:])
            # scalar's last two credit slots: stage-2 weights for expert 0
            nc.scalar.dma_start(w2_sb[0][0][:], w2[0, 0])
            nc.scalar.dma_start(w2_sb[0][1][:], w2[0, 1])
            # remaining w13[0] slabs: ht1/ht3 on sync, ht2 on gpsimd
            nc.sync.dma_start(w13_sb[0][1][:], w13[0, :, HTC:2 * HTC])
            nc.gpsimd.dma_start(w13_sb[0][2][:], w13[0, :, 2 * HTC:3 * HTC])
            nc.sync.dma_start(w13_sb[0][3][:], w13[0, :, 3 * HTC:4 * HTC])
            nc.sync.dma_start(xt_sb[1][:], xt_ds[1].ap()[:])
            nc.sync.dma_start(xt_sb[2][:], xt_ds[2].ap()[:])
            nc.sync.dma_start(xt_sb[3][:], xt_ds[3].ap()[:])
            nc.sync.dma_start(w2_sb[1][0][:], w2[1, 0])
            nc.sync.dma_start(w2_sb[1][1][:], w2[1, 1])
            # gpsimd: expert-1 stage-1 weights + the small tail block
            for ht in range(NH):
                nc.gpsimd.dma_start(w13_sb[1][ht][:],
                                    w13[1, :, ht * HTC:(ht + 1) * HTC])
            nc.gpsimd.dma_start(xt_sb[4][:], xt_ds[4].ap()[:])

            # ---- compute ----
            for bi, (e, off, n) in enumerate(XBLOCKS):
                h_all = hpool.tile([128, NH * 512], FP8, tag="h", name="h")
                for ht in range(NH):
                    g = ps1.tile([128, 512], F32, tag="g", name="g")
                    u = ps1.tile([128, 512], F32, tag="u", name="u")
                    for w in range(2):
                        dst = g if w == 0 else u
                        for j in range(ND // 2):
                            o = (w * ND + 2 * j) * 128
                            nc.tensor.matmul(
                                dst[:, :n],
                                _pair(w13_sb[e][ht][:, o: o + 256]),
                                _pair(xt_sb[bi][:, 2 * j * n: (2 * j + 2) * n]),
                                start=(j == 0), stop=(j == ND // 2 - 1),
                                perf_mode=DR,
                            )
                    sg = hpool.tile([128, 512], BF, tag="sg", name="sg")
                    nc.scalar.activation(sg[:, :n], g[:, :n], AF.Silu,
                                         scale=1.0 / S1)
                    nc.vector.tensor_mul(h_all[:, ht * 512: ht * 512 + n],
                                         sg[:, :n], u[:, :n])
                for tt in range(n // 128):
                    gtt = (SLOT_OFF[e] + off) // 128 + tt
                    ys = ypool.tile([128, D], BF, tag="ys", name="ys")
                    for db in range(2):
                        yp = ps2.tile([128, 512], F32, tag="yp", name="yp")
                        for hp in range(NH // 2):
                            nc.tensor.matmul(
                                yp[:],
                                _pair(h_all[:, 2 * hp * 512:
                                            (2 * hp + 2) * 512])
                                [:, :, tt * 128:(tt + 1) * 128],
                                _pair(w2_sb[e][db][:, 2 * hp * 512:
                                                   (2 * hp + 2) * 512]),
                                start=(hp == 0), stop=(hp == NH // 2 - 1),
                                perf_mode=DR,
                            )
                        if db == 0:
                            nc.vector.tensor_scalar_mul(
                                ys[:, 0:512], yp[:], wg_sb[:, gtt:gtt + 1])
                        else:
                            nc.scalar.mul(ys[:, 512:1024], yp[:],
                                          wg_sb[:, gtt:gtt + 1])
                    if bi == len(XBLOCKS) - 1:
                        # final tile: split across two idle rings to shorten
                        # the drain tail
                        nc.sync.dma_start(y[gtt, :, 0:512], ys[:, 0:512])
                        nc.scalar.dma_start(y[gtt, :, 512:1024],
                                            ys[:, 512:1024])
                    elif bi == len(XBLOCKS) - 2:
                        # tiles written late go on the by-then-idle sync ring
                        nc.sync.dma_start(y[gtt], ys[:])
                    else:
                        nc.gpsimd.dma_start(y[gtt], ys[:])

    nc.compile()
    return nc


def _program():
    global _PROG
    if _PROG is None:
        _PROG = _build_program()
    return _PROG


def _route(x, gate_w):
    """fp32 softmax router + top-2 with renormalized weights (matches ref)."""
    logits = x @ gate_w.astype(np.float32)
    logits = logits - logits.max(axis=-1, keepdims=True)
    ex = np.exp(logits)
    scores = ex / ex.sum(axis=-1, keepdims=True)
    idx = np.argsort(-scores, axis=-1, kind="stable")[:, :TOPK]
    w = np.take_along_axis(scores, idx, axis=-1)
    w = w / w.sum(axis=-1, keepdims=True)
    return idx, w.astype(np.float32)


def _moe_numpy(x, gate_w, w1, w3, w2):
    """Slow exact fallback (only used if a capacity overflow ever happens)."""
    idx, wts = _route(x, gate_w)
    out = x.copy()
    for e in range(E):
        sel = np.nonzero(idx == e)
        toks = sel[0]
        ww = wts[sel]
        xe = x[toks]
        g = xe @ w1[e]
        u = xe @ w3[e]
        h = (g / (1.0 + np.exp(-g))) * u
        out[toks] += (h @ w2[e]) * ww[:, None]
    return out


def _quant_fp8(a, scale):
    import ml_dtypes
    return np.clip(a * scale, -FP8_MAX, FP8_MAX).astype(ml_dtypes.float8_e4m3)


def _pack_w13(a):
    """[D, 2H] -> [128, ND*2H], columns ordered (ht, w1|w3, dt, 128)."""
    r = a.reshape(ND, 128, 2, NH, 128)        # dt, p, w, ht, c
    r = r.transpose(1, 3, 2, 0, 4)            # p, ht, w, dt, c
    return np.ascontiguousarray(r.reshape(128, ND * H2))


def _pmajor(a, cols):
    """[rows=nd*128, cols] -> [128, nd*cols] partition-major layout."""
    nd = a.shape[0] // 128
    return np.ascontiguousarray(
        a.reshape(nd, 128, cols).transpose(1, 0, 2).reshape(128, nd * cols))


def kernel(hidden_states, gate_w, w1, w3, w2):
    from concourse import bass_utils

    hidden_states = np.asarray(hidden_states, dtype=np.float32)
    gate_w = np.asarray(gate_w, dtype=np.float32)
    w1 = np.asarray(w1, dtype=np.float32)
    w3 = np.asarray(w3, dtype=np.float32)
    w2 = np.asarray(w2, dtype=np.float32)

    x = hidden_states.reshape(T, D)
    idx, wts = _route(x, gate_w)

    tok_lists = []
    wt_lists = []
    for e in range(E):
        sel = np.nonzero(idx == e)
        tok_lists.append(sel[0])
        wt_lists.append(wts[sel])
    counts = np.array([len(t) for t in tok_lists])

    # pair largest with smallest; slot A = larger of the pair
    order = np.argsort(-counts, kind="stable")
    pairs = [(order[i], order[E - 1 - i]) for i in range(NCORES)]
    if any(counts[a] > SLOT_CAP[0] or counts[b] > SLOT_CAP[1]
           for a, b in pairs):
        return _moe_numpy(x, gate_w, w1, w3, w2).reshape(B, S, D)

    xq = _quant_fp8(x, SX)                                    # [T, D] fp8
    w13q = _quant_fp8(np.concatenate([w1, w3], axis=2), SW1)  # [E, D, 2H]
    w13q = np.stack([_pack_w13(w13q[e]) for e in range(E)])
    w2q = _quant_fp8(w2, SW2)
    # [128, NH*D] (ht,d)-major -> [2, 128, NH*512] db-half-major
    w2q = np.stack([
        np.ascontiguousarray(
            _pmajor(w2q[e], D).reshape(128, NH, 2, 512)
            .transpose(2, 0, 1, 3).reshape(2, 128, NH * 512))
        for e in range(E)])

    in_maps = []
    for c in range(NCORES):
        xg = np.zeros((TOT, D), dtype=xq.dtype)
        wgt = np.zeros(TOT, dtype=np.float32)
        for j, e in enumerate(pairs[c]):
            ne = counts[e]
            xg[SLOT_OFF[j]:SLOT_OFF[j] + ne] = xq[tok_lists[e]]
            wgt[SLOT_OFF[j]:SLOT_OFF[j] + ne] = wt_lists[e] / S2
        xgT = np.ascontiguousarray(xg.T)       # [D, TOT]
        ea, eb = pairs[c]
        m = {
            "w13": np.stack([w13q[ea], w13q[eb]]),
            "w2": np.stack([w2q[ea], w2q[eb]]),
            "wg": np.ascontiguousarray(wgt.reshape(NTT, 128).T),
        }
        for bi, (s, off, n) in enumerate(XBLOCKS):
            c0 = SLOT_OFF[s] + off
            m[f"xtb{bi}"] = _pmajor(xgT[:, c0:c0 + n], n)
        in_maps.append(m)

    res = bass_utils.run_bass_kernel_spmd(
        _program(), in_maps, core_ids=list(range(NCORES)))
    global _LAST_RESULTS
    _LAST_RESULTS = res

    out = x.copy()
    for c in range(NCORES):
        yc = np.asarray(res.results[c]["y"], dtype=np.float32)
        yc = yc.reshape(NTT * 128, D)
        for j, e in enumerate(pairs[c]):
            ne = counts[e]
            out[tok_lists[e]] += yc[SLOT_OFF[j]:SLOT_OFF[j] + ne]
    return out.reshape(B, S, D)
